# revision 41
# baseline (speedup 1.0000x reference)
"""Multi-head factorized dense attention on 8 TRN2 NeuronCores.

Reference computation (per batch b):
    V = x @ Wv                      (4096, 256)
    l = x @ Wl, r = x @ Wr          (4096, 64) each
    attn[n, p*64+q] = l[n,p]*r[n,q] (4096, 4096)
    score = softmax(attn, -1)
    o = score @ V                   (shared across heads == plain matmul)
    out = o @ Wo
Sharding: 8 cores = 2 batches x 4 query-row chunks of 1024 rows.

Small O(S*D^2) projections run on the host; the device does the O(S*S)
work.  Per 128-row query tile the device computes prod = l' x r (outer
product, fp16, l' host-prescaled by 8*log2e so prod is in eighth-octave
log2 units), then produces fp8 E along four column ranges in parallel:
  - ACT (2 instrs, split at the transpose-half boundary): exact exp via
    activation(scale=1/C8, bias=negmx) -> fp8
  - DVE u16 Schraudolph: u16 = rne(prod*128 + d16) whose bits ARE the
    fp16 of e^(x-mx+5.4) (HW float->uint converts round-to-nearest and
    saturate negatives to 0); then a DVE 2x tensor_copy fp16->fp8
  - DVE + Pool u8 Schraudolph: u8 = rne(max(prod + d8, 0)) whose bits
    ARE the e4m3 of the same value (coarser: only used for a capped
    fraction of columns to stay inside the error budget)
The DVE outer-product chunks use a duplicated-pair l' layout
(lp2[p,2a+c] = l'[p,a]) so every operand has a packed 2-byte last dim
-> DVE 2x mode.  E8 then follows the baseline path: XBAR fp16-pair
transpose, fp8 DoubleRow matmuls with V8/dV8 stationary + 1/64-ones
Z-accumulator (all three PSUM accumulators share one bank so the
epilogue is a single PSUM->SBUF copy), fp16 o^T+Z shipped; host
divides by Z and applies Wo.  DMA is spread across queues: transposes
on SP, v8 split SP/Pool, out stores on ACT/Pool after the tile's exp.
"""

import sys

sys.path.insert(0, "/opt/trn_rl_repo")

import numpy as np
import ml_dtypes

B, S, D = 2, 4096, 256
PD = 64  # proj_dim_l == proj_dim_r == 64, PD*PD == S
NQ = S // 4  # query rows per core
QT = NQ // 128  # query tiles per core (8)
NJ = S // 256  # DoubleRow j-blocks (16)
N_CORES = 8
BIAS_OFF = 5.4  # exp bias: top element = e^BIAS_OFF = 221 < 240 (e4m3 max)
ZSCALE = 1.0 / 64.0  # ones-column scale: keeps Z within fp16 range
LOG2E = 1.4426950408889634
C8 = 8.0 * LOG2E  # host prescale on l: prod = 8*log2(e^(l*r))

# exp column split: [ACT exact | ACT exact | DVE u16 | DVE u8 | Pool u8]
ACT_A = 2048  # first transpose half: ACT only
ACT_B = 384  # cols 2048:2432
D16_COLS = 384  # cols 2432:2816  u16 trick (DVE) + fp8 convert (DVE)
D8_COLS = 0  # DVE u8 trick
P8_COLS = S - ACT_A - ACT_B - D16_COLS - D8_COLS  # 1024 Pool u8 trick
# outer-product a-split: DVE gets [0, MA), Pool gets [MA, 64)
MA = 46

F8NP = ml_dtypes.float8_e4m3

_CACHE = {}


def _build(nloop=0):
    if ("nc", nloop) in _CACHE:
        return _CACHE[("nc", nloop)]

    import concourse.bass as bass
    import concourse.bacc as bacc
    import concourse.tile as tile
    from concourse import mybir

    F32 = mybir.dt.float32
    F16 = mybir.dt.float16
    F8 = mybir.dt.float8e4
    U16 = mybir.dt.uint16
    U8 = mybir.dt.uint8
    EXP = mybir.ActivationFunctionType.Exp
    ADD = mybir.AluOpType.add
    MAX = mybir.AluOpType.max
    MULT = mybir.AluOpType.mult
    DR = mybir.MatmulPerfMode.DoubleRow

    nc = bacc.Bacc("TRN2", target_bir_lowering=False, debug=False)

    # cols [0:40]: aux (20 f32: d8[8], d16[8], negmx fp16 pairs[4]);
    # tile t at cols [40+192t : 40+192(t+1)]: [0:128] dup-pair l', [128:192] r
    lr_d = nc.dram_tensor("lr16", [128, 40 + QT * 192], F16, kind="ExternalInput").ap()
    v8_d = nc.dram_tensor(
        "v8", [128, NJ, 2, 2, 2, 128], F8, kind="ExternalInput"
    ).ap()
    out_d = nc.dram_tensor("out", [NQ, 3 * 128], F16, kind="ExternalOutput").ap()

    with tile.TileContext(nc) as tc:
        import contextlib

        with contextlib.ExitStack() as ctx:
            if nloop:
                ctx.enter_context(tc.For_i(0, nloop, 1))
            persist = ctx.enter_context(tc.tile_pool(name="persist", bufs=1))
            prodp = ctx.enter_context(tc.tile_pool(name="prodp", bufs=5))
            ep = ctx.enter_context(tc.tile_pool(name="ep", bufs=5))
            ebp = ctx.enter_context(tc.tile_pool(name="ebp", bufs=3))
            etp = ctx.enter_context(tc.tile_pool(name="etp", bufs=8))
            work = ctx.enter_context(tc.tile_pool(name="work", bufs=3))
            psO = ctx.enter_context(tc.tile_pool(name="psO", bufs=2, space="PSUM"))

            lr16 = persist.tile([128, 40 + QT * 192], F16, tag="lr16")
            aux = lr16[:].bitcast(F32)  # [128, 788]; aux at cols [0:20]
            v8 = persist.tile([128, NJ, 2, 2, 2, 128], F8, tag="v8")
            ones8 = persist.tile([128, 2, 128], F8, tag="ones8")
            dmy = persist.tile([128, 1], F16, tag="dmy")
            dmy8 = persist.tile([128, 1], F8, tag="dmy8")



            # warm the ACT exp table while loads run; build ones on-device
            nc.vector.memset(dmy[:], 0.0)
            nc.vector.memset(ones8[:], ZSCALE)
            nc.scalar.activation(out=dmy8[:], in_=dmy[:], func=EXP, bias=0.0, scale=1.0)

            prod_t = {}
            e8_t = {}
            et_t = {}
            ops_t = {}

            def outer(t, chunks=((0, MA, "v"), (MA, PD, "g"))):
                lp2 = lr16[:, 40 + 192 * t : 40 + 192 * t + 128]  # dup pairs
                prod = prodp.tile([128, PD, PD], F16, tag="prod", name=f"prod{t}")
                prod_t[t] = prod
                pf = prod[:].rearrange("p a b -> p (a b)")
                for a0, a1, eng in chunks:
                    w = a1 - a0
                    l_b = bass.AP(
                        tensor=lp2.tensor,
                        offset=lp2.offset + 2 * a0,
                        ap=[lp2.ap[0], [2, w], [0, 32], [1, 2]],
                    )
                    r_ap = lr16[:, 40 + 192 * t + 128 : 40 + 192 * t + 192]
                    r_b = bass.AP(
                        tensor=r_ap.tensor,
                        offset=r_ap.offset,
                        ap=[r_ap.ap[0], [0, w], [2, 32], [1, 2]],
                    )
                    o_b = bass.AP(
                        tensor=pf.tensor,
                        offset=pf.offset + 64 * a0,
                        ap=[pf.ap[0], [64, w], [2, 32], [1, 2]],
                    )
                    e = nc.vector if eng == "v" else nc.gpsimd
                    e.tensor_tensor(out=o_b, in0=l_b, in1=r_b, op=MULT)

            def expf(t, act_widths=(ACT_A, ACT_B)):
                E8 = ep.tile([128, S], F8, tag="E8", name=f"E8{t}")
                e8_t[t] = E8
                pflat = prod_t[t][:].rearrange("p a b -> p (a b)")
                E8u = E8[:].bitcast(U8)
                negmx = lr16[:, 32 + t : 33 + t]
                d8 = aux[:, t : t + 1]
                d16 = aux[:, 8 + t : 9 + t]
                c0 = 0
                for w in act_widths:
                    nc.scalar.activation(
                        out=E8[:, c0 : c0 + w],
                        in_=pflat[:, c0 : c0 + w],
                        func=EXP,
                        bias=negmx,
                        scale=1.0 / C8,
                    )
                    c0 += w
                if D16_COLS:
                    eb = ebp.tile([128, D16_COLS], U16, tag="eb", name=f"eb{t}")
                    nc.vector.tensor_scalar(
                        out=eb[:],
                        in0=pflat[:, c0 : c0 + D16_COLS],
                        scalar1=128.0,
                        scalar2=d16,
                        op0=MULT,
                        op1=ADD,
                    )
                    nc.vector.tensor_copy(
                        E8[:, c0 : c0 + D16_COLS], eb[:].bitcast(F16)
                    )
                    c0 += D16_COLS
                if D8_COLS:
                    nc.vector.tensor_scalar(
                        out=E8u[:, c0 : c0 + D8_COLS],
                        in0=pflat[:, c0 : c0 + D8_COLS],
                        scalar1=d8,
                        scalar2=0.0,
                        op0=ADD,
                        op1=MAX,
                    )
                    c0 += D8_COLS
                nc.gpsimd.tensor_scalar(
                    out=E8u[:, c0:S],
                    in0=pflat[:, c0:S],
                    scalar1=d8,
                    scalar2=0.0,
                    op0=ADD,
                    op1=MAX,
                )

            def xbar(t, j0, nj):
                E16 = e8_t[t][:].bitcast(F16)  # [128, 2048]
                half = bass.AP(
                    tensor=E16.tensor,
                    offset=E16.offset + j0 * 128,
                    ap=[E16.ap[0], [1, nj * 128]],
                )
                et = etp.tile([128, nj, 128], F16, tag="et", name=f"et{t}_{j0}")
                et_t.setdefault(t, []).append((j0, nj, et))
                nc.sync.dma_start(out=et[:], in_=half, transpose=True)

            def back(t, j0, nj):
                if j0 == 0:
                    # [128, 3, 512] f32 = 3 PSUM banks; accumulator i lives in
                    # bank i (each bank its own zero-region) at cols [i, 0:128]
                    ops_t[t] = psO.tile([128, 3, 512], F32, tag="ops", name=f"ops_{t}")
                ops = ops_t[t]
                for jj in range(nj):
                    j = j0 + jj
                    x0, _, et = next(e for e in et_t[t] if e[0] <= j < e[0] + e[1])
                    et8 = et[:].bitcast(F8)
                    # moving operand: interleaved (c, n) bytes of E^T pairs
                    x = bass.AP(
                        tensor=et8.tensor,
                        offset=et8.offset + (j - x0) * 256,
                        ap=[et8.ap[0], [1, 2], [2, 128]],
                    )
                    for w in range(2):
                        for h in range(2):
                            nc.tensor.matmul(
                                ops[:, h, 0:128],
                                v8[:, j, w, :, h, :],
                                x,
                                start=(j == 0 and w == 0),
                                stop=(j == NJ - 1 and w == 1),
                                perf_mode=DR,
                            )
                    nc.tensor.matmul(
                        ops[:, 2, 0:128],
                        ones8[:],
                        x,
                        start=(j == 0),
                        stop=(j == NJ - 1),
                        perf_mode=DR,
                    )

            osb_t = {}

            def epi(t):
                ops = ops_t[t]
                osb = work.tile([128, 3, 128], F16, tag="osb", name=f"osb{t}")
                osb_t[t] = osb
                # single merged PSUM->SBUF copy (DVE)
                nc.vector.tensor_copy(osb[:], ops[:, :, 0:128])

            def out_dma(t, eng):
                tsl = slice(t * 128, (t + 1) * 128)
                eng.dma_start(out=out_d[tsl, :], in_=osb_t[t][:])

            # ---- loads + software pipeline ----
            # one DMA brings aux + tile-0 l'/r; Pool's early v8 DMAs are
            # harmless because DVE covers all the mult chunks that gate the
            # first exp instructions (Pool only owns tile-0's tail columns)
            nc.sync.dma_start(out=lr16[:, 0:232], in_=lr_d[:, 0:232])
            nc.gpsimd.dma_start(out=lr16[:, 232:1576], in_=lr_d[:, 232:1576])
            nc.sync.dma_start(out=v8[:, 0:2], in_=v8_d[:, 0:2])
            nc.sync.dma_start(out=v8[:, 2:4], in_=v8_d[:, 2:4])

            # tile 0: fine-grained start so ACT begins ASAP; DVE owns a0-48
            # so Pool's (scheduler-hoisted) v8 loads gate nothing urgent
            outer(0, chunks=((0, 16, "v"), (16, 32, "v"), (32, 48, "v"), (48, PD, "g")))
            expf(0, act_widths=(512, 512, 1024, ACT_B))
            outer(1)
            nc.sync.dma_start(out=v8[:, 4:8], in_=v8_d[:, 4:8])
            expf(1)
            xbar(0, 0, 4)
            xbar(0, 4, 4)
            xbar(0, 8, 8)
            nc.sync.dma_start(out=v8[:, 8:12], in_=v8_d[:, 8:12])
            back(0, 0, 2)
            back(0, 2, 2)
            back(0, 4, 4)
            nc.gpsimd.dma_start(out=v8[:, 12:16], in_=v8_d[:, 12:16])
            back(0, 8, 4)
            back(0, 12, 4)
            for t in range(2, QT):
                outer(t)
                expf(t)
                xbar(t - 1, 0, 8)
                xbar(t - 1, 8, 8)
                epi(t - 2)
                out_dma(t - 2, nc.sync)
                back(t - 1, 0, 8)
                back(t - 1, 8, 8)
            xbar(QT - 1, 0, 8)
            epi(QT - 2)
            out_dma(QT - 2, nc.sync)
            back(QT - 1, 0, 8)
            # tile 7 tail: progressively smaller units
            xbar(QT - 1, 8, 4)
            back(QT - 1, 8, 4)
            xbar(QT - 1, 12, 4)
            back(QT - 1, 12, 4)
            epi(QT - 1)
            out_dma(QT - 1, nc.sync)

    nc.compile()
    _CACHE[("nc", nloop)] = nc
    return nc


def _in_maps(x, Wl, Wr, Wv, Wo):
    x = np.ascontiguousarray(x, np.float32)

    v8s = []
    l16s = []
    r16s = []
    for b in range(B):
        V = x[b] @ np.asarray(Wv, np.float32)
        V8 = V.astype(F8NP)
        dV8 = (V - V8.astype(np.float32)).astype(F8NP)
        v8 = np.zeros((128, NJ, 2, 2, 2, 128), F8NP)
        p2 = np.arange(128)
        for j in range(NJ):
            for c in range(2):
                rows = j * 256 + 2 * p2 + c
                for h in range(2):
                    v8[:, j, 0, c, h, :] = V8[rows, h * 128 : (h + 1) * 128]
                    v8[:, j, 1, c, h, :] = dV8[rows, h * 128 : (h + 1) * 128]
        v8s.append(v8)
        l16s.append((x[b] @ np.asarray(Wl, np.float32)).astype(np.float16))
        r16s.append((x[b] @ np.asarray(Wr, np.float32)).astype(np.float16))

    maps = []
    for c in range(N_CORES):
        b, q = c // 4, (c % 4) * NQ
        l16 = l16s[b][q : q + NQ].reshape(QT, 128, PD)
        r16 = r16s[b][q : q + NQ].reshape(QT, 128, PD)
        # prescaled l' (fp16) and duplicated-pair layout
        lp16 = (l16.astype(np.float32) * np.float32(C8)).astype(np.float16)
        lp2 = np.repeat(lp16, 2, axis=2)  # [QT, 128, 128]
        lrt = np.concatenate([lp2, r16], axis=2).transpose(1, 0, 2)  # [128, QT, 192]

        # row max of fp16(l'*r): max of rounded == round(max) (monotone)
        lf = lp16.astype(np.float32)
        rf = r16.astype(np.float32)
        corners = np.stack(
            [
                lf.max(2) * rf.max(2),
                lf.max(2) * rf.min(2),
                lf.min(2) * rf.max(2),
                lf.min(2) * rf.min(2),
            ],
            axis=0,
        ).max(0)
        mx8 = corners.astype(np.float16).astype(np.float32)  # [QT, 128] (8*log2 units)
        # natural-log bias for the ACT path: exp(prod/C8 + negmx)
        negmx = (-mx8 / np.float32(C8) + np.float32(BIAS_OFF)).T.astype(
            np.float16
        )  # [128, QT]
        nmf = negmx.astype(np.float32)
        # trick biases built from the fp16-ROUNDED negmx so all column
        # groups share one bias exactly (HW converts are RNE: no +0.5)
        d8 = (np.float32(56.0) + np.float32(C8) * nmf).astype(np.float32)
        d16 = (np.float32(8192.0) + np.float32(128.0) * d8).astype(np.float32)
        aux = np.zeros((128, 20), np.float32)
        aux[:, 0:8] = d8
        aux[:, 8:16] = d16
        aux[:, 16:20] = np.ascontiguousarray(negmx).view(np.float32)
        lr16 = np.zeros((128, 40 + QT * 192), np.float16)
        lr16[:, 0:40] = aux.view(np.float16)
        lr16[:, 40:] = lrt.reshape(128, QT * 192)
        maps.append(
            {
                "lr16": np.ascontiguousarray(lr16),
                "v8": v8s[b],
            }
        )
    return maps


def _finish(res_core, Wo):
    """Host epilogue for one core: reassemble o^T, normalize by Z, apply Wo."""
    arr = res_core.astype(np.float32)  # [NQ, 384]: per-tile o^T halves + Z rows
    out = np.empty((NQ, D), np.float32)
    for t in range(QT):
        blk = arr[t * 128 : (t + 1) * 128]
        o_un = np.concatenate([blk[:, 0:128].T, blk[:, 128:256].T], axis=1)
        Z = blk[0, 256:384] / np.float32(ZSCALE)
        out[t * 128 : (t + 1) * 128] = o_un / Z[:, None]
    return out @ np.asarray(Wo, np.float32)


def kernel(x, Wl, Wr, Wv, Wo, _trace=False, _result_holder=None):
    from concourse.bass_utils import run_bass_kernel_spmd

    nc = _build()
    maps = _in_maps(x, Wl, Wr, Wv, Wo)
    res = run_bass_kernel_spmd(nc, maps, list(range(N_CORES)), trace=_trace)
    if _result_holder is not None:
        _result_holder.append(res)
    out = np.empty((B, S, D), np.float32)
    for c in range(N_CORES):
        b, q = c // 4, (c % 4) * NQ
        out[b, q : q + NQ] = _finish(res.results[c]["out"], Wo)
    return out


# revision 47
# speedup vs baseline: 1.0003x; 1.0003x over previous
"""Multi-head factorized dense attention on 8 TRN2 NeuronCores.

Reference computation (per batch b):
    V = x @ Wv                      (4096, 256)
    l = x @ Wl, r = x @ Wr          (4096, 64) each
    attn[n, p*64+q] = l[n,p]*r[n,q] (4096, 4096)
    score = softmax(attn, -1)
    o = score @ V                   (shared across heads == plain matmul)
    out = o @ Wo
Sharding: 8 cores = 2 batches x 4 query-row chunks of 1024 rows.

Small O(S*D^2) projections run on the host; the device does the O(S*S)
work.  Per 128-row query tile the device computes prod = l' x r (outer
product, fp16, l' host-prescaled by 8*log2e so prod is in eighth-octave
log2 units), then produces fp8 E along four column ranges in parallel:
  - ACT (2 instrs, split at the transpose-half boundary): exact exp via
    activation(scale=1/C8, bias=negmx) -> fp8
  - DVE u16 Schraudolph: u16 = rne(prod*128 + d16) whose bits ARE the
    fp16 of e^(x-mx+5.4) (HW float->uint converts round-to-nearest and
    saturate negatives to 0); then a DVE 2x tensor_copy fp16->fp8
  - DVE + Pool u8 Schraudolph: u8 = rne(max(prod + d8, 0)) whose bits
    ARE the e4m3 of the same value (coarser: only used for a capped
    fraction of columns to stay inside the error budget)
The DVE outer-product chunks use a duplicated-pair l' layout
(lp2[p,2a+c] = l'[p,a]) so every operand has a packed 2-byte last dim
-> DVE 2x mode.  E8 then follows the baseline path: XBAR fp16-pair
transpose, fp8 DoubleRow matmuls with V8/dV8 stationary + 1/64-ones
Z-accumulator (all three PSUM accumulators share one bank so the
epilogue is a single PSUM->SBUF copy), fp16 o^T+Z shipped; host
divides by Z and applies Wo.  DMA is spread across queues: transposes
on SP, v8 split SP/Pool, out stores on ACT/Pool after the tile's exp.
"""

import sys

sys.path.insert(0, "/opt/trn_rl_repo")

import numpy as np
import ml_dtypes

B, S, D = 2, 4096, 256
PD = 64  # proj_dim_l == proj_dim_r == 64, PD*PD == S
NQ = S // 4  # query rows per core
QT = NQ // 128  # query tiles per core (8)
NJ = S // 256  # DoubleRow j-blocks (16)
N_CORES = 8
BIAS_OFF = 5.4  # exp bias: top element = e^BIAS_OFF = 221 < 240 (e4m3 max)
ZSCALE = 1.0 / 64.0  # ones-column scale: keeps Z within fp16 range
LOG2E = 1.4426950408889634
C8 = 8.0 * LOG2E  # host prescale on l: prod = 8*log2(e^(l*r))

# exp column split: [ACT exact | ACT exact | DVE u16 | DVE u8 | Pool u8]
ACT_A = 2048  # first transpose half: ACT only
ACT_B = 384  # cols 2048:2432
D16_COLS = 384  # cols 2432:2816  u16 trick (DVE) + fp8 convert (DVE)
D8_COLS = 0  # DVE u8 trick
P8_COLS = S - ACT_A - ACT_B - D16_COLS - D8_COLS  # 1024 Pool u8 trick
# outer-product a-split: DVE gets [0, MA), Pool gets [MA, 64)
MA = 46

F8NP = ml_dtypes.float8_e4m3

_CACHE = {}


def _build(nloop=0):
    if ("nc", nloop) in _CACHE:
        return _CACHE[("nc", nloop)]

    import concourse.bass as bass
    import concourse.bacc as bacc
    import concourse.tile as tile
    from concourse import mybir

    F32 = mybir.dt.float32
    F16 = mybir.dt.float16
    F8 = mybir.dt.float8e4
    U16 = mybir.dt.uint16
    U8 = mybir.dt.uint8
    EXP = mybir.ActivationFunctionType.Exp
    ADD = mybir.AluOpType.add
    MAX = mybir.AluOpType.max
    MULT = mybir.AluOpType.mult
    DR = mybir.MatmulPerfMode.DoubleRow

    nc = bacc.Bacc("TRN2", target_bir_lowering=False, debug=False)

    # cols [0:40]: aux (20 f32: d8[8], d16[8], negmx fp16 pairs[4]);
    # tile t at cols [40+192t : 40+192(t+1)]: [0:128] dup-pair l', [128:192] r
    lr_d = nc.dram_tensor("lr16", [128, 40 + QT * 192], F16, kind="ExternalInput").ap()
    v8_d = nc.dram_tensor(
        "v8", [128, NJ, 2, 2, 2, 128], F8, kind="ExternalInput"
    ).ap()
    out_d = nc.dram_tensor("out", [NQ, 3 * 128], F16, kind="ExternalOutput").ap()

    with tile.TileContext(nc) as tc:
        import contextlib

        with contextlib.ExitStack() as ctx:
            if nloop:
                ctx.enter_context(tc.For_i(0, nloop, 1))
            persist = ctx.enter_context(tc.tile_pool(name="persist", bufs=1))
            prodp = ctx.enter_context(tc.tile_pool(name="prodp", bufs=5))
            ep = ctx.enter_context(tc.tile_pool(name="ep", bufs=5))
            ebp = ctx.enter_context(tc.tile_pool(name="ebp", bufs=3))
            etp = ctx.enter_context(tc.tile_pool(name="etp", bufs=8))
            work = ctx.enter_context(tc.tile_pool(name="work", bufs=3))
            psO = ctx.enter_context(tc.tile_pool(name="psO", bufs=2, space="PSUM"))

            lr16 = persist.tile([128, 40 + QT * 192], F16, tag="lr16")
            aux = lr16[:].bitcast(F32)  # [128, 788]; aux at cols [0:20]
            v8 = persist.tile([128, NJ, 2, 2, 2, 128], F8, tag="v8")
            ones8 = persist.tile([128, 2, 128], F8, tag="ones8")
            dmy = persist.tile([128, 1], F16, tag="dmy")
            dmy8 = persist.tile([128, 1], F8, tag="dmy8")



            # warm the ACT exp table while loads run; build ones on-device
            nc.vector.memset(dmy[:], 0.0)
            nc.vector.memset(ones8[:], ZSCALE)
            nc.scalar.activation(out=dmy8[:], in_=dmy[:], func=EXP, bias=0.0, scale=1.0)

            prod_t = {}
            e8_t = {}
            et_t = {}
            ops_t = {}

            def outer(t, chunks=((0, MA, "v"), (MA, PD, "g"))):
                lp2 = lr16[:, 40 + 192 * t : 40 + 192 * t + 128]  # dup pairs
                prod = prodp.tile([128, PD, PD], F16, tag="prod", name=f"prod{t}")
                prod_t[t] = prod
                pf = prod[:].rearrange("p a b -> p (a b)")
                for a0, a1, eng in chunks:
                    w = a1 - a0
                    l_b = bass.AP(
                        tensor=lp2.tensor,
                        offset=lp2.offset + 2 * a0,
                        ap=[lp2.ap[0], [2, w], [0, 32], [1, 2]],
                    )
                    r_ap = lr16[:, 40 + 192 * t + 128 : 40 + 192 * t + 192]
                    r_b = bass.AP(
                        tensor=r_ap.tensor,
                        offset=r_ap.offset,
                        ap=[r_ap.ap[0], [0, w], [2, 32], [1, 2]],
                    )
                    o_b = bass.AP(
                        tensor=pf.tensor,
                        offset=pf.offset + 64 * a0,
                        ap=[pf.ap[0], [64, w], [2, 32], [1, 2]],
                    )
                    e = nc.vector if eng == "v" else nc.gpsimd
                    e.tensor_tensor(out=o_b, in0=l_b, in1=r_b, op=MULT)

            def expf(t, act_widths=(ACT_A, ACT_B)):
                E8 = ep.tile([128, S], F8, tag="E8", name=f"E8{t}")
                e8_t[t] = E8
                pflat = prod_t[t][:].rearrange("p a b -> p (a b)")
                E8u = E8[:].bitcast(U8)
                negmx = lr16[:, 32 + t : 33 + t]
                d8 = aux[:, t : t + 1]
                d16 = aux[:, 8 + t : 9 + t]
                c0 = 0
                for w in act_widths:
                    nc.scalar.activation(
                        out=E8[:, c0 : c0 + w],
                        in_=pflat[:, c0 : c0 + w],
                        func=EXP,
                        bias=negmx,
                        scale=1.0 / C8,
                    )
                    c0 += w
                if D16_COLS:
                    eb = ebp.tile([128, D16_COLS], U16, tag="eb", name=f"eb{t}")
                    nc.vector.tensor_scalar(
                        out=eb[:],
                        in0=pflat[:, c0 : c0 + D16_COLS],
                        scalar1=128.0,
                        scalar2=d16,
                        op0=MULT,
                        op1=ADD,
                    )
                    nc.vector.tensor_copy(
                        E8[:, c0 : c0 + D16_COLS], eb[:].bitcast(F16)
                    )
                    c0 += D16_COLS
                if D8_COLS:
                    nc.vector.tensor_scalar(
                        out=E8u[:, c0 : c0 + D8_COLS],
                        in0=pflat[:, c0 : c0 + D8_COLS],
                        scalar1=d8,
                        scalar2=0.0,
                        op0=ADD,
                        op1=MAX,
                    )
                    c0 += D8_COLS
                nc.gpsimd.tensor_scalar(
                    out=E8u[:, c0:S],
                    in0=pflat[:, c0:S],
                    scalar1=d8,
                    scalar2=0.0,
                    op0=ADD,
                    op1=MAX,
                )

            def xbar(t, j0, nj):
                E16 = e8_t[t][:].bitcast(F16)  # [128, 2048]
                half = bass.AP(
                    tensor=E16.tensor,
                    offset=E16.offset + j0 * 128,
                    ap=[E16.ap[0], [1, nj * 128]],
                )
                et = etp.tile([128, nj, 128], F16, tag="et", name=f"et{t}_{j0}")
                et_t.setdefault(t, []).append((j0, nj, et))
                nc.sync.dma_start(out=et[:], in_=half, transpose=True)

            def back(t, j0, nj):
                if j0 == 0:
                    # [128, 3, 512] f32 = 3 PSUM banks; accumulator i lives in
                    # bank i (each bank its own zero-region) at cols [i, 0:128]
                    ops_t[t] = psO.tile([128, 3, 512], F32, tag="ops", name=f"ops_{t}")
                ops = ops_t[t]
                for jj in range(nj):
                    j = j0 + jj
                    x0, _, et = next(e for e in et_t[t] if e[0] <= j < e[0] + e[1])
                    et8 = et[:].bitcast(F8)
                    # moving operand: interleaved (c, n) bytes of E^T pairs
                    x = bass.AP(
                        tensor=et8.tensor,
                        offset=et8.offset + (j - x0) * 256,
                        ap=[et8.ap[0], [1, 2], [2, 128]],
                    )
                    for w in range(2):
                        for h in range(2):
                            nc.tensor.matmul(
                                ops[:, h, 0:128],
                                v8[:, j, w, :, h, :],
                                x,
                                start=(j == 0 and w == 0),
                                stop=(j == NJ - 1 and w == 1),
                                perf_mode=DR,
                            )
                    nc.tensor.matmul(
                        ops[:, 2, 0:128],
                        ones8[:],
                        x,
                        start=(j == 0),
                        stop=(j == NJ - 1),
                        perf_mode=DR,
                    )

            osb_t = {}

            def epi(t):
                ops = ops_t[t]
                osb = work.tile([128, 3, 128], F16, tag="osb", name=f"osb{t}")
                osb_t[t] = osb
                # single merged PSUM->SBUF copy (DVE)
                nc.vector.tensor_copy(osb[:], ops[:, :, 0:128])

            def out_dma(t, eng):
                tsl = slice(t * 128, (t + 1) * 128)
                eng.dma_start(out=out_d[tsl, :], in_=osb_t[t][:])

            # ---- loads + software pipeline ----
            # one DMA brings aux + tile-0 l'/r; Pool's early v8 DMAs are
            # harmless because DVE covers all the mult chunks that gate the
            # first exp instructions (Pool only owns tile-0's tail columns)
            nc.sync.dma_start(out=lr16[:, 0:232], in_=lr_d[:, 0:232])
            nc.gpsimd.dma_start(out=lr16[:, 232:1576], in_=lr_d[:, 232:1576])
            nc.sync.dma_start(out=v8[:, 0:2], in_=v8_d[:, 0:2])
            nc.sync.dma_start(out=v8[:, 2:4], in_=v8_d[:, 2:4])

            # tile 0: fine-grained start so ACT begins ASAP; DVE owns a0-48
            # so Pool's (scheduler-hoisted) v8 loads gate nothing urgent
            outer(0, chunks=((0, 16, "v"), (16, 32, "v"), (32, 48, "v"), (48, PD, "g")))
            expf(0, act_widths=(512, 512, 1024, ACT_B))
            outer(1)
            nc.sync.dma_start(out=v8[:, 4:8], in_=v8_d[:, 4:8])
            expf(1)
            xbar(0, 0, 2)
            xbar(0, 2, 2)
            xbar(0, 4, 4)
            xbar(0, 8, 8)
            nc.sync.dma_start(out=v8[:, 8:12], in_=v8_d[:, 8:12])
            back(0, 0, 1)
            back(0, 1, 1)
            back(0, 2, 2)
            back(0, 4, 4)
            nc.gpsimd.dma_start(out=v8[:, 12:16], in_=v8_d[:, 12:16])
            back(0, 8, 4)
            back(0, 12, 4)
            for t in range(2, QT):
                outer(t)
                expf(t)
                xbar(t - 1, 0, 8)
                xbar(t - 1, 8, 8)
                epi(t - 2)
                out_dma(t - 2, nc.sync)
                back(t - 1, 0, 8)
                back(t - 1, 8, 8)
            xbar(QT - 1, 0, 8)
            epi(QT - 2)
            out_dma(QT - 2, nc.sync)
            back(QT - 1, 0, 8)
            # tile 7 tail: progressively smaller units
            xbar(QT - 1, 8, 4)
            back(QT - 1, 8, 4)
            xbar(QT - 1, 12, 4)
            back(QT - 1, 12, 4)
            epi(QT - 1)
            out_dma(QT - 1, nc.sync)

    nc.compile()
    _CACHE[("nc", nloop)] = nc
    return nc


def _in_maps(x, Wl, Wr, Wv, Wo):
    x = np.ascontiguousarray(x, np.float32)

    v8s = []
    l16s = []
    r16s = []
    for b in range(B):
        V = x[b] @ np.asarray(Wv, np.float32)
        V8 = V.astype(F8NP)
        dV8 = (V - V8.astype(np.float32)).astype(F8NP)
        v8 = np.zeros((128, NJ, 2, 2, 2, 128), F8NP)
        p2 = np.arange(128)
        for j in range(NJ):
            for c in range(2):
                rows = j * 256 + 2 * p2 + c
                for h in range(2):
                    v8[:, j, 0, c, h, :] = V8[rows, h * 128 : (h + 1) * 128]
                    v8[:, j, 1, c, h, :] = dV8[rows, h * 128 : (h + 1) * 128]
        v8s.append(v8)
        l16s.append((x[b] @ np.asarray(Wl, np.float32)).astype(np.float16))
        r16s.append((x[b] @ np.asarray(Wr, np.float32)).astype(np.float16))

    maps = []
    for c in range(N_CORES):
        b, q = c // 4, (c % 4) * NQ
        l16 = l16s[b][q : q + NQ].reshape(QT, 128, PD)
        r16 = r16s[b][q : q + NQ].reshape(QT, 128, PD)
        # prescaled l' (fp16) and duplicated-pair layout
        lp16 = (l16.astype(np.float32) * np.float32(C8)).astype(np.float16)
        lp2 = np.repeat(lp16, 2, axis=2)  # [QT, 128, 128]
        lrt = np.concatenate([lp2, r16], axis=2).transpose(1, 0, 2)  # [128, QT, 192]

        # row max of fp16(l'*r): max of rounded == round(max) (monotone)
        lf = lp16.astype(np.float32)
        rf = r16.astype(np.float32)
        corners = np.stack(
            [
                lf.max(2) * rf.max(2),
                lf.max(2) * rf.min(2),
                lf.min(2) * rf.max(2),
                lf.min(2) * rf.min(2),
            ],
            axis=0,
        ).max(0)
        mx8 = corners.astype(np.float16).astype(np.float32)  # [QT, 128] (8*log2 units)
        # natural-log bias for the ACT path: exp(prod/C8 + negmx)
        negmx = (-mx8 / np.float32(C8) + np.float32(BIAS_OFF)).T.astype(
            np.float16
        )  # [128, QT]
        nmf = negmx.astype(np.float32)
        # trick biases built from the fp16-ROUNDED negmx so all column
        # groups share one bias exactly (HW converts are RNE: no +0.5)
        d8 = (np.float32(56.0) + np.float32(C8) * nmf).astype(np.float32)
        d16 = (np.float32(8192.0) + np.float32(128.0) * d8).astype(np.float32)
        aux = np.zeros((128, 20), np.float32)
        aux[:, 0:8] = d8
        aux[:, 8:16] = d16
        aux[:, 16:20] = np.ascontiguousarray(negmx).view(np.float32)
        lr16 = np.zeros((128, 40 + QT * 192), np.float16)
        lr16[:, 0:40] = aux.view(np.float16)
        lr16[:, 40:] = lrt.reshape(128, QT * 192)
        maps.append(
            {
                "lr16": np.ascontiguousarray(lr16),
                "v8": v8s[b],
            }
        )
    return maps


def _finish(res_core, Wo):
    """Host epilogue for one core: reassemble o^T, normalize by Z, apply Wo."""
    arr = res_core.astype(np.float32)  # [NQ, 384]: per-tile o^T halves + Z rows
    out = np.empty((NQ, D), np.float32)
    for t in range(QT):
        blk = arr[t * 128 : (t + 1) * 128]
        o_un = np.concatenate([blk[:, 0:128].T, blk[:, 128:256].T], axis=1)
        Z = blk[0, 256:384] / np.float32(ZSCALE)
        out[t * 128 : (t + 1) * 128] = o_un / Z[:, None]
    return out @ np.asarray(Wo, np.float32)


def kernel(x, Wl, Wr, Wv, Wo, _trace=False, _result_holder=None):
    from concourse.bass_utils import run_bass_kernel_spmd

    nc = _build()
    maps = _in_maps(x, Wl, Wr, Wv, Wo)
    res = run_bass_kernel_spmd(nc, maps, list(range(N_CORES)), trace=_trace)
    if _result_holder is not None:
        _result_holder.append(res)
    out = np.empty((B, S, D), np.float32)
    for c in range(N_CORES):
        b, q = c // 4, (c % 4) * NQ
        out[b, q : q + NQ] = _finish(res.results[c]["out"], Wo)
    return out


# revision 60
# speedup vs baseline: 1.0241x; 1.0238x over previous
"""Multi-head factorized dense attention on 8 TRN2 NeuronCores.

Reference computation (per batch b):
    V = x @ Wv                      (4096, 256)
    l = x @ Wl, r = x @ Wr          (4096, 64) each
    attn[n, p*64+q] = l[n,p]*r[n,q] (4096, 4096)
    score = softmax(attn, -1)
    o = score @ V                   (shared across heads == plain matmul)
    out = o @ Wo
Sharding: 8 cores = 2 batches x 4 query-row chunks of 1024 rows.

Small O(S*D^2) projections run on the host; the device does the O(S*S)
work.  Per 128-row query tile the device computes prod = l' x r (outer
product, fp16, l' host-prescaled by 8*log2e so prod is in eighth-octave
log2 units), then produces fp8 E along four column ranges in parallel:
  - ACT (2 instrs, split at the transpose-half boundary): exact exp via
    activation(scale=1/C8, bias=negmx) -> fp8
  - DVE u16 Schraudolph: u16 = rne(prod*128 + d16) whose bits ARE the
    fp16 of e^(x-mx+5.4) (HW float->uint converts round-to-nearest and
    saturate negatives to 0); then a DVE 2x tensor_copy fp16->fp8
  - DVE + Pool u8 Schraudolph: u8 = rne(max(prod + d8, 0)) whose bits
    ARE the e4m3 of the same value (coarser: only used for a capped
    fraction of columns to stay inside the error budget)
The DVE outer-product chunks use a duplicated-pair l' layout
(lp2[p,2a+c] = l'[p,a]) so every operand has a packed 2-byte last dim
-> DVE 2x mode.  E8 then follows the baseline path: XBAR fp16-pair
transpose, fp8 DoubleRow matmuls with V8/dV8 stationary + 1/64-ones
Z-accumulator (all three PSUM accumulators share one bank so the
epilogue is a single PSUM->SBUF copy), fp16 o^T+Z shipped; host
divides by Z and applies Wo.  DMA is spread across queues: transposes
on SP, v8 split SP/Pool, out stores on ACT/Pool after the tile's exp.
"""

import sys

sys.path.insert(0, "/opt/trn_rl_repo")

import numpy as np
import ml_dtypes

B, S, D = 2, 4096, 256
PD = 64  # proj_dim_l == proj_dim_r == 64, PD*PD == S
NQ = S // 4  # query rows per core
QT = NQ // 128  # query tiles per core (8)
NJ = S // 256  # DoubleRow j-blocks (16)
N_CORES = 8
BIAS_OFF = 5.4  # exp bias: top element = e^BIAS_OFF = 221 < 240 (e4m3 max)
ZSCALE = 1.0 / 64.0  # ones-column scale: keeps Z within fp16 range
LOG2E = 1.4426950408889634
C8 = 8.0 * LOG2E  # host prescale on l: prod = 8*log2(e^(l*r))

# exp column split: [ACT exact | ACT exact | DVE u16 | DVE u8 | Pool u8]
ACT_A = 2048  # first transpose half: ACT only
ACT_B = 320  # cols 2048:2368
D16_COLS = 448  # cols 2368:2816 u16 trick (DVE) + convert
D8_COLS = 0  # DVE u8 trick
P8_COLS = S - ACT_A - ACT_B - D16_COLS - D8_COLS  # 1024 Pool u8 trick
# outer-product a-split: DVE gets [0, MA), Pool gets [MA, 64)
MA = 46

F8NP = ml_dtypes.float8_e4m3

_CACHE = {}


def _build(nloop=0):
    if ("nc", nloop) in _CACHE:
        return _CACHE[("nc", nloop)]

    import concourse.bass as bass
    import concourse.bacc as bacc
    import concourse.tile as tile
    from concourse import mybir

    F32 = mybir.dt.float32
    F16 = mybir.dt.float16
    F8 = mybir.dt.float8e4
    U16 = mybir.dt.uint16
    U8 = mybir.dt.uint8
    EXP = mybir.ActivationFunctionType.Exp
    ADD = mybir.AluOpType.add
    MAX = mybir.AluOpType.max
    MULT = mybir.AluOpType.mult
    DR = mybir.MatmulPerfMode.DoubleRow

    nc = bacc.Bacc("TRN2", target_bir_lowering=False, debug=False)

    # cols [0:40]: aux (20 f32: d8[8], d16[8], negmx fp16 pairs[4]);
    # tile t at cols [40+192t : 40+192(t+1)]: [0:128] dup-pair l', [128:192] r
    lr_d = nc.dram_tensor("lr16", [128, 40 + QT * 192], F16, kind="ExternalInput").ap()
    v8_d = nc.dram_tensor(
        "v8", [128, NJ, 2, 2, 2, 128], F8, kind="ExternalInput"
    ).ap()
    out_d = nc.dram_tensor("out", [NQ, 3 * 128], F16, kind="ExternalOutput").ap()

    with tile.TileContext(nc) as tc:
        import contextlib

        with contextlib.ExitStack() as ctx:
            if nloop:
                ctx.enter_context(tc.For_i(0, nloop, 1))
            persist = ctx.enter_context(tc.tile_pool(name="persist", bufs=1))
            prodp = ctx.enter_context(tc.tile_pool(name="prodp", bufs=5))
            ep = ctx.enter_context(tc.tile_pool(name="ep", bufs=5))
            ebp = ctx.enter_context(tc.tile_pool(name="ebp", bufs=3))
            etp = ctx.enter_context(tc.tile_pool(name="etp", bufs=8))
            work = ctx.enter_context(tc.tile_pool(name="work", bufs=3))
            psO = ctx.enter_context(tc.tile_pool(name="psO", bufs=2, space="PSUM"))
            psW = ctx.enter_context(tc.tile_pool(name="psW", bufs=1, space="PSUM"))

            lr16 = persist.tile([128, 40 + QT * 192], F16, tag="lr16")
            aux = lr16[:].bitcast(F32)  # [128, 788]; aux at cols [0:20]
            v8 = persist.tile([128, NJ, 2, 2, 2, 128], F8, tag="v8")
            ones8 = persist.tile([128, 2, 128], F8, tag="ones8")
            dmy = persist.tile([128, 1], F16, tag="dmy")
            dmy8 = persist.tile([128, 1], F8, tag="dmy8")



            # warm the ACT exp table while loads run; build ones on-device
            nc.vector.memset(dmy[:], 0.0)
            nc.vector.memset(ones8[:], ZSCALE)
            nc.scalar.activation(out=dmy8[:], in_=dmy[:], func=EXP, bias=0.0, scale=1.0)

            # warm the PE p-state ramp: a few dummy matmuls right after the
            # ones8 memset so the 3us clock ramp elapses before back(0)
            dps = psW.tile([128, 128], F32, tag="dps")
            o8 = ones8[:]
            xw = bass.AP(tensor=o8.tensor, offset=o8.offset, ap=[o8.ap[0], [1, 2], [2, 128]])
            for _ in range(240):
                nc.tensor.matmul(dps[:], ones8[:], xw, start=True, stop=True, perf_mode=DR)

            prod_t = {}
            e8_t = {}
            et_t = {}
            ops_t = {}

            def outer(t, chunks=((0, MA, "v"), (MA, PD, "g"))):
                lp2 = lr16[:, 40 + 192 * t : 40 + 192 * t + 128]  # dup pairs
                prod = prodp.tile([128, PD, PD], F16, tag="prod", name=f"prod{t}")
                prod_t[t] = prod
                pf = prod[:].rearrange("p a b -> p (a b)")
                for a0, a1, eng in chunks:
                    w = a1 - a0
                    l_b = bass.AP(
                        tensor=lp2.tensor,
                        offset=lp2.offset + 2 * a0,
                        ap=[lp2.ap[0], [2, w], [0, 32], [1, 2]],
                    )
                    r_ap = lr16[:, 40 + 192 * t + 128 : 40 + 192 * t + 192]
                    r_b = bass.AP(
                        tensor=r_ap.tensor,
                        offset=r_ap.offset,
                        ap=[r_ap.ap[0], [0, w], [2, 32], [1, 2]],
                    )
                    o_b = bass.AP(
                        tensor=pf.tensor,
                        offset=pf.offset + 64 * a0,
                        ap=[pf.ap[0], [64, w], [2, 32], [1, 2]],
                    )
                    e = nc.vector if eng == "v" else nc.gpsimd
                    e.tensor_tensor(out=o_b, in0=l_b, in1=r_b, op=MULT)

            def expf(t, act_widths=(ACT_A, ACT_B)):
                E8 = ep.tile([128, S], F8, tag="E8", name=f"E8{t}")
                e8_t[t] = E8
                pflat = prod_t[t][:].rearrange("p a b -> p (a b)")
                E8u = E8[:].bitcast(U8)
                negmx = lr16[:, 32 + t : 33 + t]
                d8 = aux[:, t : t + 1]
                d16 = aux[:, 8 + t : 9 + t]
                c0 = 0
                for w in act_widths:
                    nc.scalar.activation(
                        out=E8[:, c0 : c0 + w],
                        in_=pflat[:, c0 : c0 + w],
                        func=EXP,
                        bias=negmx,
                        scale=1.0 / C8,
                    )
                    c0 += w
                if D16_COLS:
                    eb = ebp.tile([128, D16_COLS], U16, tag="eb", name=f"eb{t}")
                    nc.vector.tensor_scalar(
                        out=eb[:],
                        in0=pflat[:, c0 : c0 + D16_COLS],
                        scalar1=128.0,
                        scalar2=d16,
                        op0=MULT,
                        op1=ADD,
                    )
                    nc.vector.tensor_copy(
                        E8[:, c0 : c0 + D16_COLS], eb[:].bitcast(F16)
                    )
                    c0 += D16_COLS
                if D8_COLS:
                    nc.vector.tensor_scalar(
                        out=E8u[:, c0 : c0 + D8_COLS],
                        in0=pflat[:, c0 : c0 + D8_COLS],
                        scalar1=d8,
                        scalar2=0.0,
                        op0=ADD,
                        op1=MAX,
                    )
                    c0 += D8_COLS
                nc.gpsimd.tensor_scalar(
                    out=E8u[:, c0:S],
                    in0=pflat[:, c0:S],
                    scalar1=d8,
                    scalar2=0.0,
                    op0=ADD,
                    op1=MAX,
                )

            def xbar(t, j0, nj):
                E16 = e8_t[t][:].bitcast(F16)  # [128, 2048]
                half = bass.AP(
                    tensor=E16.tensor,
                    offset=E16.offset + j0 * 128,
                    ap=[E16.ap[0], [1, nj * 128]],
                )
                et = etp.tile([128, nj, 128], F16, tag="et", name=f"et{t}_{j0}")
                et_t.setdefault(t, []).append((j0, nj, et))
                nc.sync.dma_start(out=et[:], in_=half, transpose=True)

            def back(t, j0, nj):
                if j0 == 0:
                    # [128, 3, 512] f32 = 3 PSUM banks; accumulator i lives in
                    # bank i (each bank its own zero-region) at cols [i, 0:128]
                    ops_t[t] = psO.tile([128, 3, 512], F32, tag="ops", name=f"ops_{t}")
                ops = ops_t[t]
                for jj in range(nj):
                    j = j0 + jj
                    x0, _, et = next(e for e in et_t[t] if e[0] <= j < e[0] + e[1])
                    et8 = et[:].bitcast(F8)
                    # moving operand: interleaved (c, n) bytes of E^T pairs
                    x = bass.AP(
                        tensor=et8.tensor,
                        offset=et8.offset + (j - x0) * 256,
                        ap=[et8.ap[0], [1, 2], [2, 128]],
                    )
                    for w in range(2):
                        for h in range(2):
                            nc.tensor.matmul(
                                ops[:, h, 0:128],
                                v8[:, j, w, :, h, :],
                                x,
                                start=(j == 0 and w == 0),
                                stop=(j == NJ - 1 and w == 1),
                                perf_mode=DR,
                            )
                    nc.tensor.matmul(
                        ops[:, 2, 0:128],
                        ones8[:],
                        x,
                        start=(j == 0),
                        stop=(j == NJ - 1),
                        perf_mode=DR,
                    )

            osb_t = {}

            def epi(t):
                ops = ops_t[t]
                osb = work.tile([128, 3, 128], F16, tag="osb", name=f"osb{t}")
                osb_t[t] = osb
                # single merged PSUM->SBUF copy (DVE)
                nc.vector.tensor_copy(osb[:], ops[:, :, 0:128])

            def out_dma(t, eng):
                tsl = slice(t * 128, (t + 1) * 128)
                eng.dma_start(out=out_d[tsl, :], in_=osb_t[t][:])

            # ---- loads + software pipeline ----
            # one DMA brings aux + tile-0 l'/r; Pool's early v8 DMAs are
            # harmless because DVE covers all the mult chunks that gate the
            # first exp instructions (Pool only owns tile-0's tail columns)
            nc.sync.dma_start(out=lr16[:, 0:232], in_=lr_d[:, 0:232])
            nc.gpsimd.dma_start(out=lr16[:, 232:1576], in_=lr_d[:, 232:1576])
            nc.sync.dma_start(out=v8[:, 0:2], in_=v8_d[:, 0:2])
            nc.sync.dma_start(out=v8[:, 2:4], in_=v8_d[:, 2:4])

            # tile 0: fine-grained start so ACT begins ASAP; DVE owns a0-48
            # so Pool's (scheduler-hoisted) v8 loads gate nothing urgent
            outer(0, chunks=((0, 16, "v"), (16, 32, "v"), (32, 48, "v"), (48, PD, "g")))
            expf(0, act_widths=(512, 512, 1024, ACT_B))
            outer(1)
            nc.sync.dma_start(out=v8[:, 4:8], in_=v8_d[:, 4:8])
            expf(1)
            xbar(0, 0, 2)
            xbar(0, 2, 2)
            xbar(0, 4, 4)
            xbar(0, 8, 8)
            nc.sync.dma_start(out=v8[:, 8:12], in_=v8_d[:, 8:12])
            back(0, 0, 1)
            back(0, 1, 1)
            back(0, 2, 2)
            back(0, 4, 4)
            nc.gpsimd.dma_start(out=v8[:, 12:16], in_=v8_d[:, 12:16])
            back(0, 8, 4)
            back(0, 12, 4)
            for t in range(2, QT):
                outer(t)
                expf(t)
                xbar(t - 1, 0, 8)
                xbar(t - 1, 8, 8)
                epi(t - 2)
                out_dma(t - 2, nc.sync)
                back(t - 1, 0, 8)
                back(t - 1, 8, 8)
            xbar(QT - 1, 0, 8)
            epi(QT - 2)
            out_dma(QT - 2, nc.sync)
            back(QT - 1, 0, 8)
            # tile 7 tail: progressively smaller units
            xbar(QT - 1, 8, 4)
            back(QT - 1, 8, 4)
            xbar(QT - 1, 12, 4)
            back(QT - 1, 12, 4)
            epi(QT - 1)
            out_dma(QT - 1, nc.sync)

    nc.compile()
    _CACHE[("nc", nloop)] = nc
    return nc


def _in_maps(x, Wl, Wr, Wv, Wo):
    x = np.ascontiguousarray(x, np.float32)

    v8s = []
    l16s = []
    r16s = []
    for b in range(B):
        V = x[b] @ np.asarray(Wv, np.float32)
        V8 = V.astype(F8NP)
        dV8 = (V - V8.astype(np.float32)).astype(F8NP)
        v8 = np.zeros((128, NJ, 2, 2, 2, 128), F8NP)
        p2 = np.arange(128)
        for j in range(NJ):
            for c in range(2):
                rows = j * 256 + 2 * p2 + c
                for h in range(2):
                    v8[:, j, 0, c, h, :] = V8[rows, h * 128 : (h + 1) * 128]
                    v8[:, j, 1, c, h, :] = dV8[rows, h * 128 : (h + 1) * 128]
        v8s.append(v8)
        l16s.append((x[b] @ np.asarray(Wl, np.float32)).astype(np.float16))
        r16s.append((x[b] @ np.asarray(Wr, np.float32)).astype(np.float16))

    maps = []
    for c in range(N_CORES):
        b, q = c // 4, (c % 4) * NQ
        l16 = l16s[b][q : q + NQ].reshape(QT, 128, PD)
        r16 = r16s[b][q : q + NQ].reshape(QT, 128, PD)
        # prescaled l' (fp16) and duplicated-pair layout
        lp16 = (l16.astype(np.float32) * np.float32(C8)).astype(np.float16)
        lp2 = np.repeat(lp16, 2, axis=2)  # [QT, 128, 128]
        lrt = np.concatenate([lp2, r16], axis=2).transpose(1, 0, 2)  # [128, QT, 192]

        # row max of fp16(l'*r): max of rounded == round(max) (monotone)
        lf = lp16.astype(np.float32)
        rf = r16.astype(np.float32)
        corners = np.stack(
            [
                lf.max(2) * rf.max(2),
                lf.max(2) * rf.min(2),
                lf.min(2) * rf.max(2),
                lf.min(2) * rf.min(2),
            ],
            axis=0,
        ).max(0)
        mx8 = corners.astype(np.float16).astype(np.float32)  # [QT, 128] (8*log2 units)
        # natural-log bias for the ACT path: exp(prod/C8 + negmx)
        negmx = (-mx8 / np.float32(C8) + np.float32(BIAS_OFF)).T.astype(
            np.float16
        )  # [128, QT]
        nmf = negmx.astype(np.float32)
        # trick biases built from the fp16-ROUNDED negmx so all column
        # groups share one bias exactly (HW converts are RNE: no +0.5)
        d8 = (np.float32(56.0) + np.float32(C8) * nmf).astype(np.float32)
        d16 = (np.float32(8192.0) + np.float32(128.0) * d8).astype(np.float32)
        aux = np.zeros((128, 20), np.float32)
        aux[:, 0:8] = d8
        aux[:, 8:16] = d16
        aux[:, 16:20] = np.ascontiguousarray(negmx).view(np.float32)
        lr16 = np.zeros((128, 40 + QT * 192), np.float16)
        lr16[:, 0:40] = aux.view(np.float16)
        lr16[:, 40:] = lrt.reshape(128, QT * 192)
        maps.append(
            {
                "lr16": np.ascontiguousarray(lr16),
                "v8": v8s[b],
            }
        )
    return maps


def _finish(res_core, Wo):
    """Host epilogue for one core: reassemble o^T, normalize by Z, apply Wo."""
    arr = res_core.astype(np.float32)  # [NQ, 384]: per-tile o^T halves + Z rows
    out = np.empty((NQ, D), np.float32)
    for t in range(QT):
        blk = arr[t * 128 : (t + 1) * 128]
        o_un = np.concatenate([blk[:, 0:128].T, blk[:, 128:256].T], axis=1)
        Z = blk[0, 256:384] / np.float32(ZSCALE)
        out[t * 128 : (t + 1) * 128] = o_un / Z[:, None]
    return out @ np.asarray(Wo, np.float32)


def kernel(x, Wl, Wr, Wv, Wo, _trace=False, _result_holder=None):
    from concourse.bass_utils import run_bass_kernel_spmd

    nc = _build()
    maps = _in_maps(x, Wl, Wr, Wv, Wo)
    res = run_bass_kernel_spmd(nc, maps, list(range(N_CORES)), trace=_trace)
    if _result_holder is not None:
        _result_holder.append(res)
    out = np.empty((B, S, D), np.float32)
    for c in range(N_CORES):
        b, q = c // 4, (c % 4) * NQ
        out[b, q : q + NQ] = _finish(res.results[c]["out"], Wo)
    return out
